# revision 56
# baseline (speedup 1.0000x reference)
"""Trainium2 Bass kernel for nn_AEFIN (FFT top-k masking + attention + FAN/MLP).

Data-parallel over batch: 64 batches sharded 8-per-core across 8 NeuronCores.
Inside each core (all shapes hardcoded for BS=64, L=512, E=64, pred=512):

  per pair of batches (c_pair = 2*64 = 128 channels packed on partitions):
    rfft as f32 matmul (exact enough for top-k selection)   [PE, f32]
    mag^2 -> 8-at-a-time max + match_replace top-k zap      [DVE]
    mask = (zap==0); masked spectra in bf16                 [DVE]
    irfft as bf16 matmul -> x_filt (both layouts), norm     [PE]
  per batch: single-head attention with host-folded weights [PE/ACT, bf16]
  core-wide: FAN (sin/cos/gelu) + 2-layer MLP               [PE/ACT, bf16]

Host-side folds: attention score matrix Wm = (wq/8)^T wk, k-bias via
per-key additive bias (q-side bias terms cancel in softmax), v/out_proj
merged (wvo = out_w @ wv, bias folded through attn row-sum), FAN gate
sigmoid folded into fc1 columns, all weight transposes done on host.
"""

import math
import os
import sys

for _p in ("/opt/trn_rl_repo",):
    if _p not in sys.path and os.path.isdir(_p):
        sys.path.append(_p)

import numpy as np
import ml_dtypes

BF = ml_dtypes.bfloat16
SEQ, PRED, E, BS = 512, 512, 64, 64
F = SEQ // 2 + 1  # 257
NCORES = 8
BPC = BS // NCORES       # batches per core = 8
NPAIR = BPC // 2         # 4


def _host_dft():
    n = np.arange(SEQ, dtype=np.float64)
    f = np.arange(F, dtype=np.float64)
    ang = 2.0 * np.pi * np.outer(n, f) / SEQ            # [512, 257]
    CrT = np.cos(ang).astype(np.float32)
    SiT = (-np.sin(ang)).astype(np.float32)
    w = np.full(F, 2.0 / SEQ)
    w[0] = 1.0 / SEQ
    w[-1] = 1.0 / SEQ
    angT = ang.T                                        # [257, 512]
    ArT = (w[:, None] * np.cos(angT)).astype(BF)
    AiT = (-(w[:, None]) * np.sin(angT)).astype(BF)
    return CrT, SiT, ArT, AiT


def _mk_layout(entries):
    off, c = {}, 0
    for name, w in entries:
        off[name] = (c, c + w)
        c += w
    return off, c


# f32 constant blob columns: f32 identity + FAN/MLP biases (crt/sit are
# separate params so the rfft-critical data arrives in the first DMAs)
FOFF, FW = _mk_layout([
    ("idf", 128), ("bpc", 1), ("bps", 1), ("bg", 2), ("b1", 24), ("b2", 4),
])
# bf16 constant blob columns: attention consts, FAN weights, inverse DFT
BOFF, BW = _mk_layout([
    ("wm", 128), ("wvo", 130), ("idb", 128), ("rv", 2),
    ("wpt", 4 * 128), ("wgt", 4 * 256), ("art", 3 * SEQ), ("ait", 3 * SEQ),
    ("vb", 130), ("one", 128),
])

_BUILD_CACHE = {}


def _build(k):
    """Build the SPMD Bass graph (identical on all cores). Returns nc."""
    import concourse.mybir as mybir
    import concourse.tile as tile
    from concourse import bacc
    from contextlib import ExitStack

    f32 = mybir.dt.float32
    bf16 = mybir.dt.bfloat16
    AF = mybir.ActivationFunctionType
    ALU = mybir.AluOpType

    nc = bacc.Bacc("TRN2", target_bir_lowering=False)

    # ---- DRAM parameters (per-core) ----
    P = {}

    def dparam(name, shape, dt):
        P[name] = nc.declare_dram_parameter(name, list(shape), dt, isOutput=False)
        return P[name]

    x_d = dparam("x", [BPC, SEQ, E], f32)
    crt_d = dparam("crt", [128, 4 * F], f32)
    sit_d = dparam("sit", [128, 4 * F], f32)
    blobf_d = dparam("blobf", [128, FW], f32)
    blobb_d = dparam("blobb", [128, BW], bf16)
    fc1_d = dparam("fc1t", [1024, 3072], bf16)
    fc2_d = dparam("fc2t", [3072, 512], bf16)
    out_d = nc.declare_dram_parameter("out", [2, BPC, SEQ, E], f32, isOutput=True)

    rounds = []
    rem = k
    while rem > 0:
        rounds.append(min(8, rem))
        rem -= 8

    with tile.TileContext(nc) as tc, ExitStack() as ctx:
        sb = ctx.enter_context(tc.tile_pool(name="sb", bufs=1))
        ps = ctx.enter_context(tc.tile_pool(name="ps", bufs=1, space="PSUM"))

        def st(shape, dt, tag, bufs=1):
            return sb.tile(shape, dt, tag=tag, bufs=bufs, name=tag)

        def pt(shape, tag, bufs, dt=None):
            return ps.tile(shape, dt or mybir.dt.float32, tag=tag, bufs=bufs,
                           name=tag)

        dma = nc.sync.dma_start

        # ---- constant blobs: one DMA each (SP DMA issue is ~600ns per
        # instruction, so small constants are packed host-side) ----
        crtb = st([128, 4 * F], f32, "crtb")
        for c in range(4):
            dma(out=crtb[:, c * F:(c + 1) * F], in_=crt_d.ap()[:, c * F:(c + 1) * F])
        sitb = st([128, 4 * F], f32, "sitb")
        blobf = st([128, FW], f32, "blobf")
        blobb = st([128, BW], bf16, "blobb")

        def fview(lo, hi):
            return blobf[:, lo:hi]

        crt = crtb.rearrange("p (a f) -> p a f", f=F)
        sit = sitb.rearrange("p (a f) -> p a f", f=F)
        identf = fview(*FOFF["idf"])
        bpcs = fview(*FOFF["bpc"])
        bpss = fview(*FOFF["bps"])
        bg2 = fview(*FOFF["bg"])
        b1 = fview(*FOFF["b1"])
        b2 = fview(*FOFF["b2"])

        def bview(lo, hi):
            return blobb[:, lo:hi]

        wm = bview(*BOFF["wm"])
        wvo = bview(*BOFF["wvo"])
        rvec = bview(*BOFF["rv"])
        identb = bview(*BOFF["idb"])
        wpt = bview(*BOFF["wpt"]).rearrange("p (a m) -> p a m", m=128)
        wgt = bview(*BOFF["wgt"]).rearrange("p (a m) -> p a m", m=256)
        art = bview(*BOFF["art"]).rearrange("p (a m) -> p a m", m=SEQ)
        ait = bview(*BOFF["ait"]).rearrange("p (a m) -> p a m", m=SEQ)
        vbias = blobb[0:1, BOFF["vb"][0]:BOFF["vb"][1]]
        ones1 = blobb[0:1, BOFF["one"][0]:BOFF["one"][1]]

        # prefetch all per-pair inputs up front (8 small DMAs)
        xps = []
        for p in range(NPAIR):
            xp = st([128, 4, 128], f32, "xp", bufs=NPAIR)
            for j in range(2):
                dma(out=xp[:, :, j * 64:(j + 1) * 64],
                    in_=x_d.ap()[2 * p + j].rearrange("(l p) c -> p l c", p=128))
            xps.append(xp)
            if p == 0:
                dma(out=sitb, in_=sit_d.ap())
        dma(out=blobb, in_=blobb_d.ap())
        dma(out=blobf, in_=blobf_d.ap())

        # core-wide activations (c_all = 512 columns = 8 batches x 64 ch)
        filt = st([128, 4, 512], bf16, "filt")      # x_filt, L-major chunks
        xbf = st([128, 4, 512], bf16, "xbf")        # x cast, L-major chunks
        ht = st([128, 4, 512], bf16, "ht")          # fan features (cos,sin,g0,g1)

        # ---- PE warm-up: junk matmuls while the input DMAs stream, so the
        # HAM clock-gate opens (K=8/8, 2.4 GHz) before the first real matmul
        wup = st([128, 640], bf16, "wup")
        nc.vector.memset(wup, 1.0)
        for i in range(18):
            w_ps = pt([128, 512], "pbig", 3)
            nc.tensor.matmul(w_ps, wup[:, 0:128], wup[:, 128:640],
                             start=True, stop=True)

        # ---- attention emitters (interleaved into the MLP phase below in
        # bulk stages: long runs of same-kind matmuls keep the PE stream
        # dense so the HAM clock-gate stays open, and the ACT exp latency is
        # hidden behind interleaved z1 chunks) ----
        def attn_stage_a(p):
            # Pair-stacked operands: channel rows 0:64 = batch 2p, 64:128 =
            # 2p+1. wm is blockdiag so one K=128 matmul does both batches'
            # tT; r/wvo are zero-padded per-batch stacks so the output axis
            # separates batches.
            ftbf, ntbf = pair_data[p]
            tt_ps = pt([128, 512], "pbig", 3)
            nc.tensor.matmul(tt_ps, wm, ftbf, start=True, stop=True)
            ttbf = st([128, 512], bf16, "ttbf", bufs=2)
            nc.scalar.copy(ttbf, tt_ps)

            t0_ps = pt([128, 4, 2], "psm", 2)
            for l in range(4):
                nc.tensor.matmul(t0_ps[:, l, :],
                                 ntbf[:, l * 128:(l + 1) * 128], rvec,
                                 start=True, stop=True)
            t0sb = st([128, 4, 2], f32, "t0sb", bufs=2)
            nc.scalar.copy(t0sb, t0_ps)

            # v for both batches in one matmul per pair of L-chunks: rhs is
            # the [wvoT|0 ; 0|wvoT] block layout so output cols 0:65 are
            # batch 2p's Vcomb and 65:130 batch 2p+1's.
            v_psA = pt([128, 2, 130], "zfft", 2)
            v_psB = pt([128, 2, 130], "zfft", 2)
            for l in range(4):
                tgt = (v_psA if l < 2 else v_psB)[:, l % 2, :]
                nc.tensor.matmul(tgt, ntbf[:, l * 128:(l + 1) * 128],
                                 wvo, start=True, stop=False)
                nc.tensor.matmul(tgt, ones1, vbias, start=False, stop=True)
            vbf = st([128, 4, 130], bf16, "vbf", bufs=2)
            nc.scalar.copy(vbf[:, 0:2, :], v_psA)
            nc.scalar.copy(vbf[:, 2:4, :], v_psB)
            return (ntbf, ttbf, t0sb, vbf)

        def attn_stage_b(p, state):
            # scores for both batches interleaved so the K=64 matmuls land in
            # different PE row groups (base partitions 0 and 64) and overlap.
            ntbf, ttbf, t0sb, vbf = state
            ubfs = [st([128, 4, 512], bf16, "ubf", bufs=4) for _ in range(2)]
            for l in range(4):
                for j in range(2):
                    rows = slice(j * 64, (j + 1) * 64)
                    # alternate PSUM tags: zfft's slots are idle in this phase,
                    # so the 8 score matmuls aren't throttled by exp drains
                    sc_ps = pt([128, 512], "zfft" if j else "pbig",
                                2 if j else 3)
                    nc.tensor.matmul(sc_ps, ntbf[rows, l * 128:(l + 1) * 128],
                                     ttbf[rows, :], start=True, stop=True)
                    nc.scalar.activation(ubfs[j][:, l, :], sc_ps, AF.Exp,
                                         bias=t0sb[:, l, j:j + 1])
            return (vbf, ubfs)

        def attn_stage_c(p, state):
            vbf, ubfs = state
            for j in range(2):
                b = 2 * p + j
                aot_ps = pt([65, 512], "pbig", 3)
                for l in range(4):
                    nc.tensor.matmul(aot_ps,
                                     vbf[:, l, j * 65:(j + 1) * 65],
                                     ubfs[j][:, l, :],
                                     start=(l == 0), stop=(l == 3))
                aot = st([65, 512], f32, "aot", bufs=2)
                nc.scalar.copy(aot, aot_ps)

                aof = st([128, 4, 64], f32, "aof", bufs=2)
                for qc in range(4):
                    ao_ps = pt([128, 65], "psm", 2)
                    nc.tensor.transpose(ao_ps, aot[:, qc * 128:(qc + 1) * 128],
                                        identf[0:65, 0:65])
                    rec = st([128, 1], f32, "rec", bufs=2)
                    nc.vector.reciprocal(rec, ao_ps[:, 64:65])
                    nc.vector.tensor_scalar(aof[:, qc, :], ao_ps[:, 0:64],
                                            rec, None, op0=ALU.mult)
                dma(out=out_d.ap()[0, b].rearrange("(q p) c -> p q c", p=128),
                    in_=aof)

        def attn_group(g):
            states = [attn_stage_a(2 * g + i) for i in range(2)]
            yield
            states = [attn_stage_b(2 * g + i, s) for i, s in enumerate(states)]
            yield
            for i, s in enumerate(states):
                attn_stage_c(2 * g + i, s)

        # Keep-warm scratch: junk matmuls into one dedicated PSUM tile (the
        # z2 slot is idle until the MLP tail) so the HAM clock-gate stays
        # open through the DVE-bound top-k stretches. WAW-chained on PE only.
        jt = pt([128, 512], "z2", 1)

        # ================= per-pair FFT / top-k / irfft =================
        pair_data = []
        pending = []

        def tick():
            if pending:
                try:
                    next(pending[0])
                except StopIteration:
                    pending.pop(0)
                    tick()

        for p in range(NPAIR):
            xp = xps[p]

            zr_ps = pt([128, F], "zfft", 2)
            zi_ps = pt([128, F], "zfft", 2)
            for kc in range(4):
                nc.tensor.matmul(zr_ps, xp[:, kc, :], crt[:, kc, :],
                                 start=(kc == 0), stop=(kc == 3))
            for kc in range(4):
                nc.tensor.matmul(zi_ps, xp[:, kc, :], sit[:, kc, :],
                                 start=(kc == 0), stop=(kc == 3))

            for _ in range(6):
                nc.tensor.matmul(jt, wup[:, 0:128], wup[:, 128:640],
                                 start=True, stop=True)
            sqr = st([128, F], f32, "sqr", bufs=2)
            nc.scalar.square(sqr, zr_ps)
            sqi = st([128, F], f32, "sqi", bufs=2)
            nc.scalar.square(sqi, zi_ps)
            zrbf = st([128, F], bf16, "zrbf", bufs=2)
            nc.scalar.copy(zrbf, zr_ps)
            zibf = st([128, F], bf16, "zibf", bufs=2)
            nc.scalar.copy(zibf, zi_ps)

            zap = st([128, F], f32, "zap", bufs=2)
            nc.vector.tensor_add(zap, sqr, sqi)
            m8 = st([128, 8], f32, "m8", bufs=2)
            for take in rounds:
                nc.vector.max(out=m8, in_=zap)
                if take < 8:
                    nc.vector.memset(m8[:, take:8], 0.0)
                nc.vector.match_replace(out=zap, in_to_replace=m8,
                                        in_values=zap, imm_value=0.0)
            maskb = st([128, F], bf16, "maskb", bufs=2)
            if k > 0:
                nc.vector.tensor_scalar(maskb, zap, 0.0, None, op0=ALU.is_equal)
            else:
                nc.vector.memset(maskb, 0.0)
            zrm = st([128, F], bf16, "zrm", bufs=2)
            nc.vector.tensor_mul(zrm, zrbf, maskb)
            zim = st([128, F], bf16, "zim", bufs=2)
            nc.vector.tensor_mul(zim, zibf, maskb)

            # transpose masked spectra to F-major [257, 128]
            zmr_ps = pt([128, 384], "pbig", 3, bf16)
            zmi_ps = pt([128, 384], "pbig", 3, bf16)
            for (src, dst) in ((zrm, zmr_ps), (zim, zmi_ps)):
                nc.tensor.transpose(dst[:, 0:128], src[:, 0:128], identb)
                nc.tensor.transpose(dst[:, 128:256], src[:, 128:256], identb)
                nc.tensor.transpose(dst[0:1, 256:384], src[:, 256:257], identb)
            zmr = st([128, 384], bf16, "zmr", bufs=2)
            nc.scalar.copy(zmr[:, 0:256], zmr_ps[:, 0:256])
            nc.scalar.copy(zmr[0:1, 256:384], zmr_ps[0:1, 256:384])
            zmi = st([128, 384], bf16, "zmi", bufs=2)
            nc.scalar.copy(zmi[:, 0:256], zmi_ps[:, 0:256])
            nc.scalar.copy(zmi[0:1, 256:384], zmi_ps[0:1, 256:384])

            # irfft -> filtT [c_pair, 512] (chan-major)
            ft_ps = pt([128, 512], "pbig", 3)
            ir_ops = [(zmr[:, 0:128], art[:, 0, :]),
                      (zmr[:, 128:256], art[:, 1, :]),
                      (zmr[0:1, 256:384], art[0:1, 2, :]),
                      (zmi[:, 0:128], ait[:, 0, :]),
                      (zmi[:, 128:256], ait[:, 1, :]),
                      (zmi[0:1, 256:384], ait[0:1, 2, :])]
            for i, (lhsT, rhs) in enumerate(ir_ops):
                nc.tensor.matmul(ft_ps, lhsT, rhs, start=(i == 0),
                                 stop=(i == len(ir_ops) - 1))
            ftbf = st([128, 512], bf16, "ftbf", bufs=NPAIR)
            nc.scalar.copy(ftbf, ft_ps)

            # xT via PE transpose; normT = xT - filtT (bf16)
            xt_ps = pt([128, 512], "pbig", 3)
            for l in range(4):
                nc.tensor.transpose(xt_ps[:, l * 128:(l + 1) * 128],
                                    xp[:, l, :], identf)
            xtsb = st([128, 512], f32, "xtsb", bufs=2)
            nc.scalar.copy(xtsb, xt_ps)
            ntbf = st([128, 512], bf16, "ntbf", bufs=NPAIR)
            nc.vector.tensor_sub(ntbf, xtsb, ft_ps)

            # filt L-major via PE transpose of filtT
            fl_ps = pt([128, 512], "pbig", 3, bf16)
            for l in range(4):
                nc.tensor.transpose(fl_ps[:, l * 128:(l + 1) * 128],
                                    ftbf[:, l * 128:(l + 1) * 128], identb)
            nc.scalar.copy(filt[:, :, p * 128:(p + 1) * 128],
                           fl_ps.rearrange("p (a b) -> p a b", b=128))
            nc.scalar.copy(xbf[:, :, p * 128:(p + 1) * 128], xp)
            pair_data.append((ftbf, ntbf))

        # ---- heavy weight loads: single strided DMAs, emitted after the
        # pair loop so their traffic doesn't queue ahead of the input loads
        fc1 = st([128, 8, 3072], bf16, "fc1")
        dma(out=fc1, in_=fc1_d.ap().rearrange("(a p) m -> p a m", p=128))
        fc2 = st([128, 24, 512], bf16, "fc2")
        dma(out=fc2, in_=fc2_d.ap().rearrange("(a p) m -> p a m", p=128))

        # ================= FAN (core-wide, 512 cols) =================
        pT_ps = pt([128, 512], "pbig", 3)
        for kc in range(4):
            nc.tensor.matmul(pT_ps, wpt[:, kc, :], filt[:, kc, :],
                             start=(kc == 0), stop=(kc == 3))
        # cos chunk via half-angle (ACT Sin is only valid on [-pi, pi]):
        # cos(p + bp) = 1 - 2*sin((p + bp)/2)^2
        shalf = st([128, 512], f32, "shalf")
        nc.scalar.activation(shalf, pT_ps, AF.Sin, bias=bpcs, scale=0.5)
        sh2 = st([128, 512], f32, "sh2")
        nc.scalar.square(sh2, shalf)
        nc.vector.tensor_scalar(ht[:, 0, :], sh2, -2.0, 1.0,
                                op0=ALU.mult, op1=ALU.add)
        nc.scalar.activation(ht[:, 1, :], pT_ps, AF.Sin, bias=bpss)
        for mc in range(2):
            g_ps = pt([128, 512], "pbig", 3)
            for kc in range(4):
                nc.tensor.matmul(g_ps, wgt[:, kc, mc * 128:(mc + 1) * 128],
                                 filt[:, kc, :], start=(kc == 0), stop=(kc == 3))
            nc.scalar.activation(ht[:, 2 + mc, :], g_ps, AF.Gelu,
                                 bias=bg2[:, mc:mc + 1])

        # ================= MLP (attention groups interleaved) ============
        pending.append(attn_group(0))
        pending.append(attn_group(1))
        attn_slots = {1, 4, 8, 12, 15, 19}

        z1rs = []
        for kc in range(24):
            z1_ps = pt([128, 512], "pbig", 3)
            for kk in range(8):
                rhs = ht[:, kk, :] if kk < 4 else xbf[:, kk - 4, :]
                nc.tensor.matmul(z1_ps, fc1[:, kk, kc * 128:(kc + 1) * 128],
                                 rhs, start=(kk == 0), stop=(kk == 7))
            z1r = st([128, 512], bf16, "z1r", bufs=24)
            nc.scalar.activation(z1r, z1_ps, AF.Relu, bias=b1[:, kc:kc + 1])
            z1rs.append(z1r)
            if kc in attn_slots:
                tick()

        for m in range(4):
            tick()
            z2_ps = pt([128, 512], "z2", 1)
            for kc in range(24):
                nc.tensor.matmul(z2_ps, fc2[:, kc, m * 128:(m + 1) * 128],
                                 z1rs[kc], start=(kc == 0), stop=(kc == 23))
            z2sb = st([128, 512], f32, "z2sb", bufs=2)
            nc.scalar.activation(z2sb, z2_ps, AF.Identity, bias=b2[:, m:m + 1])
            dma(out=out_d.ap()[1, :, m * 128:(m + 1) * 128, :]
                .rearrange("b p c -> p b c"),
                in_=z2sb.rearrange("p (b c) -> p b c", c=64))
        while pending:
            tick()

    nc.compile()
    return nc


def _host_inputs(inputs):
    """Host-side preprocessing -> dict of per-core-replicated input arrays
    (everything except 'x', which is sharded)."""
    f32 = np.float32
    in_proj_w = np.asarray(inputs["in_proj_w"], f32)
    in_proj_b = np.asarray(inputs["in_proj_b"], f32)
    wq, wk, wv = np.split(in_proj_w, 3, 0)
    bq, bk, bv = np.split(in_proj_b, 3, 0)
    out_w = np.asarray(inputs["out_w"], f32)
    out_b = np.asarray(inputs["out_b"], f32)

    Wm = ((wq.T / 8.0) @ wk).astype(f32)                 # [cin, cin2]
    wm2 = np.zeros((128, 128), f32)
    wm2[0:64, 0:64] = Wm
    wm2[64:128, 64:128] = Wm
    r = (wk.T @ (bq / 8.0)).astype(f32)
    wvo = out_w @ wv
    out_bp = out_b + out_w @ bv
    wvoT_ext = np.concatenate([wvo.T, np.zeros((64, 1), f32)], 1)  # [64, 65]
    wvo2 = np.zeros((128, 130), f32)
    wvo2[0:64, 0:65] = wvoT_ext
    wvo2[64:128, 65:130] = wvoT_ext
    r2 = np.zeros((128, 2), f32)
    r2[0:64, 0] = r
    r2[64:128, 1] = r
    vb = np.concatenate([out_bp, [1.0]])
    vbias_row = np.concatenate([vb, vb]).astype(BF).reshape(1, 130)

    gate = np.asarray(inputs["gate"], f32)
    gt = 1.0 / (1.0 + math.exp(-float(gate[0])))
    Wp = np.asarray(inputs["Wp"], f32)
    bp = np.asarray(inputs["bp"], f32)
    Wg = np.asarray(inputs["Wg"], f32)
    bg = np.asarray(inputs["bg"], f32)
    fc1_w = np.asarray(inputs["fc1_w"], f32)
    fc1_b = np.asarray(inputs["fc1_b"], f32)
    fc2_w = np.asarray(inputs["fc2_w"], f32)
    fc2_b = np.asarray(inputs["fc2_b"], f32)
    colscale = np.concatenate([
        np.full(128, gt), np.full(128, gt), np.full(256, 1.0 - gt), np.ones(512)
    ]).astype(f32)

    CrT, SiT, ArT, AiT = _host_dft()

    def chunked(mat, nch, width):
        """[nch*128, width] -> [128, nch*width] with chunk c at cols
        c*width:(c+1)*width (rows beyond the matrix end are zero)."""
        out = np.zeros((128, nch * width), mat.dtype)
        for c in range(nch):
            rows = mat[c * 128:(c + 1) * 128]
            out[0:rows.shape[0], c * width:(c + 1) * width] = rows
        return out

    blobf = np.zeros((128, FW), f32)

    def putf(name, arr):
        lo, hi = FOFF[name]
        blobf[:, lo:hi] = arr

    putf("idf", np.eye(128, dtype=f32))
    putf("bpc", (bp / 2.0).reshape(128, 1))
    putf("bps", bp.reshape(128, 1))
    putf("bg", bg.reshape(2, 128).T)
    putf("b1", fc1_b.reshape(24, 128).T)
    putf("b2", fc2_b.reshape(4, 128).T)

    blobb = np.zeros((128, BW), np.float32)

    def putb(name, arr):
        lo, hi = BOFF[name]
        blobb[0:arr.shape[0], lo:hi] = arr

    putb("wm", wm2)
    putb("wvo", wvo2)
    putb("idb", np.eye(128, dtype=f32))
    putb("rv", r2)
    putb("wpt", chunked(Wp.T.astype(f32), 4, 128))
    putb("wgt", chunked(Wg.T.astype(f32), 4, 256))
    putb("art", chunked(ArT.astype(f32), 3, SEQ))
    putb("ait", chunked(AiT.astype(f32), 3, SEQ))
    putb("vb", vbias_row.astype(f32))
    putb("one", np.ones((1, 128), f32))

    return {
        "crt": chunked(CrT, 4, F),
        "sit": chunked(SiT, 4, F),
        "blobf": blobf,
        "blobb": blobb.astype(BF),
        "fc1t": (fc1_w * colscale[None, :]).T.astype(BF).copy(),
        "fc2t": fc2_w.T.astype(BF).copy(),
    }


_RUN_KWARGS = {}   # test harness can set e.g. {"trace": True}
_LAST_RESULT = None


def kernel(**inputs):
    from concourse.bass_utils import run_bass_kernel_spmd

    k = int(np.asarray(inputs["freq_topk"]))
    if k not in _BUILD_CACHE:
        _BUILD_CACHE[k] = _build(k)
    nc = _BUILD_CACHE[k]

    const = _host_inputs(inputs)
    x = np.ascontiguousarray(np.asarray(inputs["batch_x"], np.float32))
    in_maps = []
    for c in range(NCORES):
        m = dict(const)
        m["x"] = np.ascontiguousarray(x[c * BPC:(c + 1) * BPC])
        in_maps.append(m)

    # occasional transient NRT_EXEC_UNIT_UNRECOVERABLE on this fleet; retry
    last_exc = None
    for attempt in range(3):
        try:
            res = run_bass_kernel_spmd(nc, in_maps,
                                       core_ids=list(range(NCORES)),
                                       **_RUN_KWARGS)
            outs = [np.asarray(res.results[c]["out"]) for c in range(NCORES)]
            globals()["_LAST_RESULT"] = res
            return np.concatenate(outs, axis=1).astype(np.float32)
        except Exception as e:  # noqa: BLE001
            last_exc = e
            import time
            time.sleep(2.0 * (attempt + 1))
    raise last_exc


if __name__ == "__main__":
    d = np.load("/tmp/ref_inputs.npz")
    inputs = {kk: d[kk] for kk in d.files}
    out = kernel(**inputs)
    ref = np.load("/tmp/ref_out.npy")
    rel = np.linalg.norm(out - ref) / np.linalg.norm(ref)
    print("rel err:", rel)


# revision 57
# speedup vs baseline: 1.0150x; 1.0150x over previous
"""Trainium2 Bass kernel for nn_AEFIN (FFT top-k masking + attention + FAN/MLP).

Data-parallel over batch: 64 batches sharded 8-per-core across 8 NeuronCores.
Inside each core (all shapes hardcoded for BS=64, L=512, E=64, pred=512):

  per pair of batches (c_pair = 2*64 = 128 channels packed on partitions):
    rfft as f32 matmul (exact enough for top-k selection)   [PE, f32]
    mag^2 -> 8-at-a-time max + match_replace top-k zap      [DVE]
    mask = (zap==0); masked spectra in bf16                 [DVE]
    irfft as bf16 matmul -> x_filt (both layouts), norm     [PE]
  per batch: single-head attention with host-folded weights [PE/ACT, bf16]
  core-wide: FAN (sin/cos/gelu) + 2-layer MLP               [PE/ACT, bf16]

Host-side folds: attention score matrix Wm = (wq/8)^T wk, k-bias via
per-key additive bias (q-side bias terms cancel in softmax), v/out_proj
merged (wvo = out_w @ wv, bias folded through attn row-sum), FAN gate
sigmoid folded into fc1 columns, all weight transposes done on host.
"""

import math
import os
import sys

for _p in ("/opt/trn_rl_repo",):
    if _p not in sys.path and os.path.isdir(_p):
        sys.path.append(_p)

import numpy as np
import ml_dtypes

BF = ml_dtypes.bfloat16
SEQ, PRED, E, BS = 512, 512, 64, 64
F = SEQ // 2 + 1  # 257
NCORES = 8
BPC = BS // NCORES       # batches per core = 8
NPAIR = BPC // 2         # 4


def _host_dft():
    n = np.arange(SEQ, dtype=np.float64)
    f = np.arange(F, dtype=np.float64)
    ang = 2.0 * np.pi * np.outer(n, f) / SEQ            # [512, 257]
    CrT = np.cos(ang).astype(np.float32)
    SiT = (-np.sin(ang)).astype(np.float32)
    w = np.full(F, 2.0 / SEQ)
    w[0] = 1.0 / SEQ
    w[-1] = 1.0 / SEQ
    angT = ang.T                                        # [257, 512]
    ArT = (w[:, None] * np.cos(angT)).astype(BF)
    AiT = (-(w[:, None]) * np.sin(angT)).astype(BF)
    return CrT, SiT, ArT, AiT


def _mk_layout(entries):
    off, c = {}, 0
    for name, w in entries:
        off[name] = (c, c + w)
        c += w
    return off, c


# f32 constant blob columns: f32 identity + FAN/MLP biases (crt/sit are
# separate params so the rfft-critical data arrives in the first DMAs)
FOFF, FW = _mk_layout([
    ("idf", 128), ("bpc", 1), ("bps", 1), ("bg", 2), ("b1", 24), ("b2", 4),
])
# bf16 constant blob columns: attention consts, FAN weights, inverse DFT
BOFF, BW = _mk_layout([
    ("wm", 128), ("wvo", 130), ("idb", 128), ("rv", 2),
    ("wpt", 4 * 128), ("wgt", 4 * 256), ("art", 3 * SEQ), ("ait", 3 * SEQ),
    ("vb", 130), ("one", 128),
])

_BUILD_CACHE = {}


def _build(k):
    """Build the SPMD Bass graph (identical on all cores). Returns nc."""
    import concourse.mybir as mybir
    import concourse.tile as tile
    from concourse import bacc
    from contextlib import ExitStack

    f32 = mybir.dt.float32
    bf16 = mybir.dt.bfloat16
    AF = mybir.ActivationFunctionType
    ALU = mybir.AluOpType

    nc = bacc.Bacc("TRN2", target_bir_lowering=False)

    # ---- DRAM parameters (per-core) ----
    P = {}

    def dparam(name, shape, dt):
        P[name] = nc.declare_dram_parameter(name, list(shape), dt, isOutput=False)
        return P[name]

    x_d = dparam("x", [BPC, SEQ, E], f32)
    crt_d = dparam("crt", [128, 4 * F], f32)
    sit_d = dparam("sit", [128, 4 * F], f32)
    blobf_d = dparam("blobf", [128, FW], f32)
    blobb_d = dparam("blobb", [128, BW], bf16)
    fc1_d = dparam("fc1t", [1024, 3072], bf16)
    fc2_d = dparam("fc2t", [3072, 512], bf16)
    out_d = nc.declare_dram_parameter("out", [2, BPC, SEQ, E], f32, isOutput=True)

    rounds = []
    rem = k
    while rem > 0:
        rounds.append(min(8, rem))
        rem -= 8

    with tile.TileContext(nc) as tc, ExitStack() as ctx:
        sb = ctx.enter_context(tc.tile_pool(name="sb", bufs=1))
        ps = ctx.enter_context(tc.tile_pool(name="ps", bufs=1, space="PSUM"))

        def st(shape, dt, tag, bufs=1):
            return sb.tile(shape, dt, tag=tag, bufs=bufs, name=tag)

        def pt(shape, tag, bufs, dt=None):
            return ps.tile(shape, dt or mybir.dt.float32, tag=tag, bufs=bufs,
                           name=tag)

        dma = nc.sync.dma_start

        # ---- constant blobs: one DMA each (SP DMA issue is ~600ns per
        # instruction, so small constants are packed host-side) ----
        crtb = st([128, 4 * F], f32, "crtb")
        for c in range(4):
            dma(out=crtb[:, c * F:(c + 1) * F], in_=crt_d.ap()[:, c * F:(c + 1) * F])
        sitb = st([128, 4 * F], f32, "sitb")
        blobf = st([128, FW], f32, "blobf")
        blobb = st([128, BW], bf16, "blobb")

        def fview(lo, hi):
            return blobf[:, lo:hi]

        crt = crtb.rearrange("p (a f) -> p a f", f=F)
        sit = sitb.rearrange("p (a f) -> p a f", f=F)
        identf = fview(*FOFF["idf"])
        bpcs = fview(*FOFF["bpc"])
        bpss = fview(*FOFF["bps"])
        bg2 = fview(*FOFF["bg"])
        b1 = fview(*FOFF["b1"])
        b2 = fview(*FOFF["b2"])

        def bview(lo, hi):
            return blobb[:, lo:hi]

        wm = bview(*BOFF["wm"])
        wvo = bview(*BOFF["wvo"])
        rvec = bview(*BOFF["rv"])
        identb = bview(*BOFF["idb"])
        wpt = bview(*BOFF["wpt"]).rearrange("p (a m) -> p a m", m=128)
        wgt = bview(*BOFF["wgt"]).rearrange("p (a m) -> p a m", m=256)
        art = bview(*BOFF["art"]).rearrange("p (a m) -> p a m", m=SEQ)
        ait = bview(*BOFF["ait"]).rearrange("p (a m) -> p a m", m=SEQ)
        vbias = blobb[0:1, BOFF["vb"][0]:BOFF["vb"][1]]
        ones1 = blobb[0:1, BOFF["one"][0]:BOFF["one"][1]]

        # prefetch all per-pair inputs up front (8 small DMAs)
        xps = []
        for p in range(NPAIR):
            xp = st([128, 4, 128], f32, "xp", bufs=NPAIR)
            for j in range(2):
                dma(out=xp[:, :, j * 64:(j + 1) * 64],
                    in_=x_d.ap()[2 * p + j].rearrange("(l p) c -> p l c", p=128))
            xps.append(xp)
            if p == 0:
                dma(out=sitb, in_=sit_d.ap())
        dma(out=blobb, in_=blobb_d.ap())
        dma(out=blobf, in_=blobf_d.ap())

        # core-wide activations (c_all = 512 columns = 8 batches x 64 ch)
        filt = st([128, 4, 512], bf16, "filt")      # x_filt, L-major chunks
        xbf = st([128, 4, 512], bf16, "xbf")        # x cast, L-major chunks
        ht = st([128, 4, 512], bf16, "ht")          # fan features (cos,sin,g0,g1)

        # ---- PE warm-up: junk matmuls while the input DMAs stream, so the
        # HAM clock-gate opens (K=8/8, 2.4 GHz) before the first real matmul
        wup = st([128, 640], bf16, "wup")
        nc.vector.memset(wup, 1.0)
        for i in range(18):
            w_ps = pt([128, 512], "pbig", 3)
            nc.tensor.matmul(w_ps, wup[:, 0:128], wup[:, 128:640],
                             start=True, stop=True)

        # ---- attention emitters (interleaved into the MLP phase below in
        # bulk stages: long runs of same-kind matmuls keep the PE stream
        # dense so the HAM clock-gate stays open, and the ACT exp latency is
        # hidden behind interleaved z1 chunks) ----
        def attn_stage_a(p):
            # Pair-stacked operands: channel rows 0:64 = batch 2p, 64:128 =
            # 2p+1. wm is blockdiag so one K=128 matmul does both batches'
            # tT; r/wvo are zero-padded per-batch stacks so the output axis
            # separates batches.
            ftbf, ntbf = pair_data[p]
            tt_ps = pt([128, 512], "pbig", 3)
            nc.tensor.matmul(tt_ps, wm, ftbf, start=True, stop=True)
            ttbf = st([128, 512], bf16, "ttbf", bufs=2)
            nc.scalar.copy(ttbf, tt_ps)

            t0_ps = pt([128, 4, 2], "psm", 2)
            for l in range(4):
                nc.tensor.matmul(t0_ps[:, l, :],
                                 ntbf[:, l * 128:(l + 1) * 128], rvec,
                                 start=True, stop=True)
            t0sb = st([128, 4, 2], f32, "t0sb", bufs=2)
            nc.scalar.copy(t0sb, t0_ps)

            # v for both batches in one matmul per pair of L-chunks: rhs is
            # the [wvoT|0 ; 0|wvoT] block layout so output cols 0:65 are
            # batch 2p's Vcomb and 65:130 batch 2p+1's.
            v_psA = pt([128, 2, 130], "zfft", 2)
            v_psB = pt([128, 2, 130], "zfft", 2)
            for l in range(4):
                tgt = (v_psA if l < 2 else v_psB)[:, l % 2, :]
                nc.tensor.matmul(tgt, ntbf[:, l * 128:(l + 1) * 128],
                                 wvo, start=True, stop=False)
                nc.tensor.matmul(tgt, ones1, vbias, start=False, stop=True)
            vbf = st([128, 4, 130], bf16, "vbf", bufs=2)
            nc.scalar.copy(vbf[:, 0:2, :], v_psA)
            nc.scalar.copy(vbf[:, 2:4, :], v_psB)
            return (ntbf, ttbf, t0sb, vbf)

        def attn_stage_b(p, state):
            # scores for both batches interleaved so the K=64 matmuls land in
            # different PE row groups (base partitions 0 and 64) and overlap.
            ntbf, ttbf, t0sb, vbf = state
            ubfs = [st([128, 4, 512], bf16, "ubf", bufs=4) for _ in range(2)]
            for l in range(4):
                for j in range(2):
                    rows = slice(j * 64, (j + 1) * 64)
                    # alternate PSUM tags: zfft's slots are idle in this phase,
                    # so the 8 score matmuls aren't throttled by exp drains
                    sc_ps = pt([128, 512], "zfft" if j else "pbig",
                                2 if j else 3)
                    nc.tensor.matmul(sc_ps, ntbf[rows, l * 128:(l + 1) * 128],
                                     ttbf[rows, :], start=True, stop=True)
                    nc.scalar.activation(ubfs[j][:, l, :], sc_ps, AF.Exp,
                                         bias=t0sb[:, l, j:j + 1])
            return (vbf, ubfs)

        def attn_stage_c(p, state):
            vbf, ubfs = state
            for j in range(2):
                b = 2 * p + j
                aot_ps = pt([65, 512], "pbig", 3)
                for l in range(4):
                    nc.tensor.matmul(aot_ps,
                                     vbf[:, l, j * 65:(j + 1) * 65],
                                     ubfs[j][:, l, :],
                                     start=(l == 0), stop=(l == 3))
                aot = st([65, 512], f32, "aot", bufs=2)
                nc.scalar.copy(aot, aot_ps)

                aof = st([128, 4, 64], f32, "aof", bufs=2)
                for qc in range(4):
                    ao_ps = pt([128, 65], "psm", 2)
                    nc.tensor.transpose(ao_ps, aot[:, qc * 128:(qc + 1) * 128],
                                        identf[0:65, 0:65])
                    rec = st([128, 1], f32, "rec", bufs=2)
                    nc.vector.reciprocal(rec, ao_ps[:, 64:65])
                    nc.vector.tensor_scalar(aof[:, qc, :], ao_ps[:, 0:64],
                                            rec, None, op0=ALU.mult)
                dma(out=out_d.ap()[0, b].rearrange("(q p) c -> p q c", p=128),
                    in_=aof)

        def attn_group(g):
            states = [attn_stage_a(2 * g + i) for i in range(2)]
            yield
            states = [attn_stage_b(2 * g + i, s) for i, s in enumerate(states)]
            yield
            for i, s in enumerate(states):
                attn_stage_c(2 * g + i, s)

        # ================= per-pair FFT / top-k / irfft =================
        pair_data = []
        pending = []

        def tick():
            if pending:
                try:
                    next(pending[0])
                except StopIteration:
                    pending.pop(0)
                    tick()

        for p in range(NPAIR):
            xp = xps[p]

            zr_ps = pt([128, F], "zfft", 2)
            zi_ps = pt([128, F], "zfft", 2)
            for kc in range(4):
                nc.tensor.matmul(zr_ps, xp[:, kc, :], crt[:, kc, :],
                                 start=(kc == 0), stop=(kc == 3))
            for kc in range(4):
                nc.tensor.matmul(zi_ps, xp[:, kc, :], sit[:, kc, :],
                                 start=(kc == 0), stop=(kc == 3))

            sqr = st([128, F], f32, "sqr", bufs=2)
            nc.scalar.square(sqr, zr_ps)
            sqi = st([128, F], f32, "sqi", bufs=2)
            nc.scalar.square(sqi, zi_ps)
            zrbf = st([128, F], bf16, "zrbf", bufs=2)
            nc.scalar.copy(zrbf, zr_ps)
            zibf = st([128, F], bf16, "zibf", bufs=2)
            nc.scalar.copy(zibf, zi_ps)

            zap = st([128, F], f32, "zap", bufs=2)
            nc.vector.tensor_add(zap, sqr, sqi)
            m8 = st([128, 8], f32, "m8", bufs=2)
            for take in rounds:
                nc.vector.max(out=m8, in_=zap)
                if take < 8:
                    nc.vector.memset(m8[:, take:8], 0.0)
                nc.vector.match_replace(out=zap, in_to_replace=m8,
                                        in_values=zap, imm_value=0.0)
            maskb = st([128, F], bf16, "maskb", bufs=2)
            if k > 0:
                nc.vector.tensor_scalar(maskb, zap, 0.0, None, op0=ALU.is_equal)
            else:
                nc.vector.memset(maskb, 0.0)
            zrm = st([128, F], bf16, "zrm", bufs=2)
            nc.vector.tensor_mul(zrm, zrbf, maskb)
            zim = st([128, F], bf16, "zim", bufs=2)
            nc.vector.tensor_mul(zim, zibf, maskb)

            # transpose masked spectra to F-major [257, 128]
            zmr_ps = pt([128, 384], "pbig", 3, bf16)
            zmi_ps = pt([128, 384], "pbig", 3, bf16)
            for (src, dst) in ((zrm, zmr_ps), (zim, zmi_ps)):
                nc.tensor.transpose(dst[:, 0:128], src[:, 0:128], identb)
                nc.tensor.transpose(dst[:, 128:256], src[:, 128:256], identb)
                nc.tensor.transpose(dst[0:1, 256:384], src[:, 256:257], identb)
            zmr = st([128, 384], bf16, "zmr", bufs=2)
            nc.scalar.copy(zmr[:, 0:256], zmr_ps[:, 0:256])
            nc.scalar.copy(zmr[0:1, 256:384], zmr_ps[0:1, 256:384])
            zmi = st([128, 384], bf16, "zmi", bufs=2)
            nc.scalar.copy(zmi[:, 0:256], zmi_ps[:, 0:256])
            nc.scalar.copy(zmi[0:1, 256:384], zmi_ps[0:1, 256:384])

            # irfft -> filtT [c_pair, 512] (chan-major)
            ft_ps = pt([128, 512], "pbig", 3)
            ir_ops = [(zmr[:, 0:128], art[:, 0, :]),
                      (zmr[:, 128:256], art[:, 1, :]),
                      (zmr[0:1, 256:384], art[0:1, 2, :]),
                      (zmi[:, 0:128], ait[:, 0, :]),
                      (zmi[:, 128:256], ait[:, 1, :]),
                      (zmi[0:1, 256:384], ait[0:1, 2, :])]
            for i, (lhsT, rhs) in enumerate(ir_ops):
                nc.tensor.matmul(ft_ps, lhsT, rhs, start=(i == 0),
                                 stop=(i == len(ir_ops) - 1))
            ftbf = st([128, 512], bf16, "ftbf", bufs=NPAIR)
            nc.scalar.copy(ftbf, ft_ps)

            # xT via PE transpose; normT = xT - filtT (bf16)
            xt_ps = pt([128, 512], "pbig", 3)
            for l in range(4):
                nc.tensor.transpose(xt_ps[:, l * 128:(l + 1) * 128],
                                    xp[:, l, :], identf)
            xtsb = st([128, 512], f32, "xtsb", bufs=2)
            nc.scalar.copy(xtsb, xt_ps)
            ntbf = st([128, 512], bf16, "ntbf", bufs=NPAIR)
            nc.vector.tensor_sub(ntbf, xtsb, ft_ps)

            # filt L-major via PE transpose of filtT
            fl_ps = pt([128, 512], "pbig", 3, bf16)
            for l in range(4):
                nc.tensor.transpose(fl_ps[:, l * 128:(l + 1) * 128],
                                    ftbf[:, l * 128:(l + 1) * 128], identb)
            nc.scalar.copy(filt[:, :, p * 128:(p + 1) * 128],
                           fl_ps.rearrange("p (a b) -> p a b", b=128))
            nc.scalar.copy(xbf[:, :, p * 128:(p + 1) * 128], xp)
            pair_data.append((ftbf, ntbf))

        # ---- heavy weight loads: single strided DMAs, emitted after the
        # pair loop so their traffic doesn't queue ahead of the input loads
        fc1 = st([128, 8, 3072], bf16, "fc1")
        dma(out=fc1, in_=fc1_d.ap().rearrange("(a p) m -> p a m", p=128))
        fc2 = st([128, 24, 512], bf16, "fc2")
        dma(out=fc2, in_=fc2_d.ap().rearrange("(a p) m -> p a m", p=128))

        # ================= FAN (core-wide, 512 cols) =================
        pT_ps = pt([128, 512], "pbig", 3)
        for kc in range(4):
            nc.tensor.matmul(pT_ps, wpt[:, kc, :], filt[:, kc, :],
                             start=(kc == 0), stop=(kc == 3))
        # cos chunk via half-angle (ACT Sin is only valid on [-pi, pi]):
        # cos(p + bp) = 1 - 2*sin((p + bp)/2)^2
        shalf = st([128, 512], f32, "shalf")
        nc.scalar.activation(shalf, pT_ps, AF.Sin, bias=bpcs, scale=0.5)
        sh2 = st([128, 512], f32, "sh2")
        nc.scalar.square(sh2, shalf)
        nc.vector.tensor_scalar(ht[:, 0, :], sh2, -2.0, 1.0,
                                op0=ALU.mult, op1=ALU.add)
        nc.scalar.activation(ht[:, 1, :], pT_ps, AF.Sin, bias=bpss)
        for mc in range(2):
            g_ps = pt([128, 512], "pbig", 3)
            for kc in range(4):
                nc.tensor.matmul(g_ps, wgt[:, kc, mc * 128:(mc + 1) * 128],
                                 filt[:, kc, :], start=(kc == 0), stop=(kc == 3))
            nc.scalar.activation(ht[:, 2 + mc, :], g_ps, AF.Gelu,
                                 bias=bg2[:, mc:mc + 1])

        # ================= MLP (attention groups interleaved) ============
        pending.append(attn_group(0))
        pending.append(attn_group(1))
        attn_slots = {1, 4, 8, 12, 15, 19}

        z1rs = []
        for kc in range(24):
            z1_ps = pt([128, 512], "pbig", 3)
            for kk in range(8):
                rhs = ht[:, kk, :] if kk < 4 else xbf[:, kk - 4, :]
                nc.tensor.matmul(z1_ps, fc1[:, kk, kc * 128:(kc + 1) * 128],
                                 rhs, start=(kk == 0), stop=(kk == 7))
            z1r = st([128, 512], bf16, "z1r", bufs=24)
            nc.scalar.activation(z1r, z1_ps, AF.Relu, bias=b1[:, kc:kc + 1])
            z1rs.append(z1r)
            if kc in attn_slots:
                tick()

        for m in range(4):
            tick()
            z2_ps = pt([128, 512], "z2", 1)
            for kc in range(24):
                nc.tensor.matmul(z2_ps, fc2[:, kc, m * 128:(m + 1) * 128],
                                 z1rs[kc], start=(kc == 0), stop=(kc == 23))
            z2sb = st([128, 512], f32, "z2sb", bufs=2)
            nc.scalar.activation(z2sb, z2_ps, AF.Identity, bias=b2[:, m:m + 1])
            dma(out=out_d.ap()[1, :, m * 128:(m + 1) * 128, :]
                .rearrange("b p c -> p b c"),
                in_=z2sb.rearrange("p (b c) -> p b c", c=64))
        while pending:
            tick()

    nc.compile()
    return nc


def _host_inputs(inputs):
    """Host-side preprocessing -> dict of per-core-replicated input arrays
    (everything except 'x', which is sharded)."""
    f32 = np.float32
    in_proj_w = np.asarray(inputs["in_proj_w"], f32)
    in_proj_b = np.asarray(inputs["in_proj_b"], f32)
    wq, wk, wv = np.split(in_proj_w, 3, 0)
    bq, bk, bv = np.split(in_proj_b, 3, 0)
    out_w = np.asarray(inputs["out_w"], f32)
    out_b = np.asarray(inputs["out_b"], f32)

    Wm = ((wq.T / 8.0) @ wk).astype(f32)                 # [cin, cin2]
    wm2 = np.zeros((128, 128), f32)
    wm2[0:64, 0:64] = Wm
    wm2[64:128, 64:128] = Wm
    r = (wk.T @ (bq / 8.0)).astype(f32)
    wvo = out_w @ wv
    out_bp = out_b + out_w @ bv
    wvoT_ext = np.concatenate([wvo.T, np.zeros((64, 1), f32)], 1)  # [64, 65]
    wvo2 = np.zeros((128, 130), f32)
    wvo2[0:64, 0:65] = wvoT_ext
    wvo2[64:128, 65:130] = wvoT_ext
    r2 = np.zeros((128, 2), f32)
    r2[0:64, 0] = r
    r2[64:128, 1] = r
    vb = np.concatenate([out_bp, [1.0]])
    vbias_row = np.concatenate([vb, vb]).astype(BF).reshape(1, 130)

    gate = np.asarray(inputs["gate"], f32)
    gt = 1.0 / (1.0 + math.exp(-float(gate[0])))
    Wp = np.asarray(inputs["Wp"], f32)
    bp = np.asarray(inputs["bp"], f32)
    Wg = np.asarray(inputs["Wg"], f32)
    bg = np.asarray(inputs["bg"], f32)
    fc1_w = np.asarray(inputs["fc1_w"], f32)
    fc1_b = np.asarray(inputs["fc1_b"], f32)
    fc2_w = np.asarray(inputs["fc2_w"], f32)
    fc2_b = np.asarray(inputs["fc2_b"], f32)
    colscale = np.concatenate([
        np.full(128, gt), np.full(128, gt), np.full(256, 1.0 - gt), np.ones(512)
    ]).astype(f32)

    CrT, SiT, ArT, AiT = _host_dft()

    def chunked(mat, nch, width):
        """[nch*128, width] -> [128, nch*width] with chunk c at cols
        c*width:(c+1)*width (rows beyond the matrix end are zero)."""
        out = np.zeros((128, nch * width), mat.dtype)
        for c in range(nch):
            rows = mat[c * 128:(c + 1) * 128]
            out[0:rows.shape[0], c * width:(c + 1) * width] = rows
        return out

    blobf = np.zeros((128, FW), f32)

    def putf(name, arr):
        lo, hi = FOFF[name]
        blobf[:, lo:hi] = arr

    putf("idf", np.eye(128, dtype=f32))
    putf("bpc", (bp / 2.0).reshape(128, 1))
    putf("bps", bp.reshape(128, 1))
    putf("bg", bg.reshape(2, 128).T)
    putf("b1", fc1_b.reshape(24, 128).T)
    putf("b2", fc2_b.reshape(4, 128).T)

    blobb = np.zeros((128, BW), np.float32)

    def putb(name, arr):
        lo, hi = BOFF[name]
        blobb[0:arr.shape[0], lo:hi] = arr

    putb("wm", wm2)
    putb("wvo", wvo2)
    putb("idb", np.eye(128, dtype=f32))
    putb("rv", r2)
    putb("wpt", chunked(Wp.T.astype(f32), 4, 128))
    putb("wgt", chunked(Wg.T.astype(f32), 4, 256))
    putb("art", chunked(ArT.astype(f32), 3, SEQ))
    putb("ait", chunked(AiT.astype(f32), 3, SEQ))
    putb("vb", vbias_row.astype(f32))
    putb("one", np.ones((1, 128), f32))

    return {
        "crt": chunked(CrT, 4, F),
        "sit": chunked(SiT, 4, F),
        "blobf": blobf,
        "blobb": blobb.astype(BF),
        "fc1t": (fc1_w * colscale[None, :]).T.astype(BF).copy(),
        "fc2t": fc2_w.T.astype(BF).copy(),
    }


_RUN_KWARGS = {}   # test harness can set e.g. {"trace": True}
_LAST_RESULT = None


def kernel(**inputs):
    from concourse.bass_utils import run_bass_kernel_spmd

    k = int(np.asarray(inputs["freq_topk"]))
    if k not in _BUILD_CACHE:
        _BUILD_CACHE[k] = _build(k)
    nc = _BUILD_CACHE[k]

    const = _host_inputs(inputs)
    x = np.ascontiguousarray(np.asarray(inputs["batch_x"], np.float32))
    in_maps = []
    for c in range(NCORES):
        m = dict(const)
        m["x"] = np.ascontiguousarray(x[c * BPC:(c + 1) * BPC])
        in_maps.append(m)

    # occasional transient NRT_EXEC_UNIT_UNRECOVERABLE on this fleet; retry
    last_exc = None
    for attempt in range(3):
        try:
            res = run_bass_kernel_spmd(nc, in_maps,
                                       core_ids=list(range(NCORES)),
                                       **_RUN_KWARGS)
            outs = [np.asarray(res.results[c]["out"]) for c in range(NCORES)]
            globals()["_LAST_RESULT"] = res
            return np.concatenate(outs, axis=1).astype(np.float32)
        except Exception as e:  # noqa: BLE001
            last_exc = e
            import time
            time.sleep(2.0 * (attempt + 1))
    raise last_exc


if __name__ == "__main__":
    d = np.load("/tmp/ref_inputs.npz")
    inputs = {kk: d[kk] for kk in d.files}
    out = kernel(**inputs)
    ref = np.load("/tmp/ref_out.npy")
    rel = np.linalg.norm(out - ref) / np.linalg.norm(ref)
    print("rel err:", rel)
